# revision 43
# baseline (speedup 1.0000x reference)
"""DeltaNet forward kernel for 8 Trainium2 NeuronCores.

Problem (hardcoded from the task spec): hidden_states [B=4, T=2048, D=1024],
H=4 heads, Dh=256, causal depthwise conv K=4 + silu on q/k/v projections,
q/k l2-normalized per head (q scaled Dh^-0.5), delta-rule recurrence over T,
per-head RMSNorm, merge heads, out = o @ Wo.

Sharding: data-parallel over (batch, head-group): core c -> batch c//2,
head group c%2 (projection columns [512*(c%2), 512*(c%2)+512)). Each core
computes a partial product against its 512 rows of Wo; the host sums the two
partials per batch (the unshard step for the row-parallel output matmul).

Device algorithm: chunked WY form of the delta rule, chunk C=128. Per chunk
(per head): KK = K K^T; R ~= (I+B)^{-1} = sum_{k<32}(-B)^k for the full
strict-upper B via Neumann doubling (I-B)(I+B^2)(I+B^4)(I+B^8)(I+B^16)
(truncation err ~1e-5, well under fp16 noise); U' = R^T (V - K S);
O = Q S + triu(K Q^T)^T U'; S += K^T U'. S accumulates in PSUM f32; matmul
operands are 16-bit.

Scheduling: all work is emitted through interleaved Python generators —
emission order sets the Tile scheduler's priorities. The two heads alternate
stage-by-stage, each head runs one chunk of lookahead (the next chunk's
S-independent work — transposes, grams, masks, doubling — fills the
sequential S-chain's stalls, gated so S-dependent stages see the right S
tile), phase A's 16 projection/l2 chains run in a staggered round-robin
window, each head's recurrence starts as soon as its projections finish, and
the output projection (o @ Wo) runs inside the chunk loop.
"""

import numpy as np

B, T, D = 4, 2048, 1024
H = 4
DH = D // H          # 256
CONV_K = 4
EPS = 1e-5
NCORES = 8
CG = 512             # columns per core (2 heads)
C = 128              # recurrence chunk length
NCHUNK = T // C      # 16
PAD = 4              # front zero padding on time axis for the causal conv
TOKB = 512           # token block (matmul moving size)
KT = D // 128        # 8 contraction tiles
CT = CG // 128       # 4 column tiles per core
NB = T // TOKB       # 4 token blocks

_CACHE = {}
SILU_NATIVE = True  # CoreSim lacks Silu; set False for simulation runs
DEBUG_SKIP_WO = False  # debug: skip output projection phase


def _build_bass():
    import concourse.bass as bass  # noqa: F401
    import concourse.bacc as bacc
    import concourse.mybir as mybir
    import concourse.tile as tile

    dt = mybir.dt
    nc = bacc.Bacc("TRN2", target_bir_lowering=False, debug=False)

    xT = nc.dram_tensor("xT", [D, T], dt.float16, kind="ExternalInput")
    wq = nc.dram_tensor("wq", [D, CG], dt.float16, kind="ExternalInput")
    wk = nc.dram_tensor("wk", [D, CG], dt.float16, kind="ExternalInput")
    wv = nc.dram_tensor("wv", [D, CG], dt.float16, kind="ExternalInput")
    wo = nc.dram_tensor("wo", [CG, D], dt.float16, kind="ExternalInput")
    cw = nc.dram_tensor("cw", [CG, 3 * CONV_K], dt.float32, kind="ExternalInput")
    consts = nc.dram_tensor("consts", [128, 6 * 128], dt.float16,
                            kind="ExternalInput")
    out = nc.dram_tensor("out", [T, D], dt.float32, kind="ExternalOutput")

    with tile.TileContext(nc) as tc:
        _body(nc, tc, mybir, xT, wq, wk, wv, wo, cw, consts, out)

    nc.compile()
    return nc


def _body(nc, tc, mybir, xT, wq, wk, wv, wo, cw, consts, out):
    dt = mybir.dt
    AF = mybir.ActivationFunctionType
    ALU = mybir.AluOpType
    fp32 = dt.float32
    bf16 = dt.float16  # 16-bit working dtype (fp16: 11-bit mantissa)
    NT = T + PAD

    xT_t = xT.ap().rearrange("(n p) t -> n p t", p=128)       # [8,128,T]
    w_t = {"q": wq.ap().rearrange("(n p) c -> n p c", p=128),
           "k": wk.ap().rearrange("(n p) c -> n p c", p=128),
           "v": wv.ap().rearrange("(n p) c -> n p c", p=128)}
    wo_t = wo.ap().rearrange("(n p) c -> n p c", p=128)       # [4,128,D]
    cw_t = cw.ap().rearrange("(n p) c -> n p c", p=128)       # [4,128,12]
    out_t = out.ap().rearrange("(n p) c -> n p c", p=128)     # [16,128,D]

    # ---------- persistent pool (lives for the whole kernel) ----------
    with tc.tile_pool(name="persist", bufs=1) as persist, \
         tc.tile_pool(name="qkvp", bufs=3 * CT) as qkvp, \
         tc.tile_pool(name="wop", bufs=CT) as wop, \
         tc.tile_pool(name="psw", bufs=6, space="PSUM") as psw, \
         tc.tile_pool(name="pss", bufs=2, space="PSUM") as pss:

        cons = persist.tile([128, 6 * 128], bf16, name="cons", tag="cons")
        nc.sync.dma_start(cons[:], consts.ap())
        ident = cons[:, 0:128]          # identity
        m_bdl = cons[:, 128:256]        # strict lower, +1
        m_bdu = cons[:, 256:384]        # strict upper, +1
        m_triuI = cons[:, 512:640]      # i<=j, +1
        ones128 = cons[:, 640:768]      # all ones

        biases = persist.tile([128, 3], dt.float32, name="biases", tag="biases")
        nc.vector.memset(biases[:, 0:1], 1e-6)
        nc.vector.memset(biases[:, 1:2], EPS)
        nc.vector.memset(biases[:, 2:3], 1e-6 * DH)
        # eps rows, added into the l2-norm sum-of-squares via a K=1 matmul
        epsrow = persist.tile([1, 2 * TOKB], bf16, name="epsrow", tag="epsrow")
        nc.vector.memset(epsrow[:, 0:TOKB], 1e-6 * DH)   # q: 256*(ss+1e-6)
        nc.vector.memset(epsrow[:, TOKB:2 * TOKB], 1e-6)  # k: ss+1e-6

        cwt = []
        for ct in range(CT):
            t_ = persist.tile([128, 3 * CONV_K], fp32, name=f"cw{ct}",
                              tag=f"cw{ct}")
            nc.sync.dma_start(t_[:], cw_t[ct])
            cwt.append(t_)

        wo_s = []
        for ct in range(CT):
            t_ = wop.tile([128, D], bf16, name=f"wo{ct}", tag="wo")
            nc.sync.dma_start(t_[:], wo_t[ct])
            wo_s.append(t_)

        qh, kh, vh = [], [], []
        for lst, nm in ((qh, "q"), (kh, "k"), (vh, "v")):
            for ct in range(CT):
                lst.append(qkvp.tile([128, T], bf16, name=f"{nm}hat{ct}",
                                     tag="qkv"))

        # ================= phase A: projections + conv + silu + l2norm ====
        with tc.tile_pool(name="xp", bufs=KT) as xp, \
             tc.tile_pool(name="wp", bufs=3 * KT) as wp, \
             tc.tile_pool(name="rawp", bufs=4) as rawp, \
             tc.tile_pool(name="sqp", bufs=4) as sqp, \
             tc.tile_pool(name="bcp", bufs=3) as bcp, \
             tc.tile_pool(name="recp", bufs=1) as recp, \
             tc.tile_pool(name="otp", bufs=8) as otp, \
             tc.tile_pool(name="ofp", bufs=4) as ofp:

            # interleave x and weight loads kt-wise so the first projection
            # matmuls (which consume kt=0 tiles first) start early
            xt = [xp.tile([128, T], bf16, name=f"xt{kt}", tag="xt")
                  for kt in range(KT)]
            ws = {nm: [wp.tile([128, CG], bf16, name=f"w{nm}{kt}", tag="w")
                       for kt in range(KT)] for nm in ("q", "k", "v")}
            for kt in range(KT):
                nc.sync.dma_start(xt[kt][:], xT_t[kt])
                for nm in ("q", "k", "v"):
                    nc.sync.dma_start(ws[nm][kt][:], w_t[nm][kt])

            sq_tiles = {}

            def proj_chain(ti, nm, dest, ct):
                """Projection + conv + silu (+ square) for one column tile."""
                rawt = rawp.tile([128, NT], bf16, name=f"raw{nm}{ct}",
                                 tag="raw")
                nc.vector.memset(rawt[:, 0:PAD], 0.0)
                dst = dest[ct]
                for nb in range(NB):
                    pt = psw.tile([128, TOKB], fp32, name=f"pp{nm}{ct}{nb}",
                                  tag="w")
                    for kt in range(KT):
                        nc.tensor.matmul(
                            pt[:], ws[nm][kt][:, ct * 128:(ct + 1) * 128],
                            xt[kt][:, nb * TOKB:(nb + 1) * TOKB],
                            start=(kt == 0), stop=(kt == KT - 1))
                    nc.scalar.copy(
                        rawt[:, PAD + nb * TOKB:PAD + (nb + 1) * TOKB],
                        pt[:])
                    yield
                # causal depthwise conv along t (tap 0 runs on ACT as a
                # per-partition-scaled copy; DVE is the phase-A bottleneck)
                w0 = cwt[ct][:, ti * CONV_K:ti * CONV_K + 1]
                nc.scalar.mul(dst[:], rawt[:, 1:1 + T], w0)
                for i in range(1, CONV_K):
                    wi = cwt[ct][:, ti * CONV_K + i:ti * CONV_K + i + 1]
                    nc.vector.scalar_tensor_tensor(
                        dst[:], rawt[:, 1 + i:1 + i + T], wi, dst[:],
                        ALU.mult, ALU.add)
                    yield
                if SILU_NATIVE:
                    nc.scalar.activation(dst[:], dst[:], AF.Silu)
                else:
                    sg = rawp.tile([128, T], bf16, name=f"sg{nm}{ct}",
                                   tag="raw")
                    nc.scalar.activation(sg[:], dst[:], AF.Sigmoid)
                    nc.vector.tensor_mul(dst[:], dst[:], sg[:])
                if ti < 2:
                    yield
                    sqt = sqp.tile([128, T], bf16, name=f"sq{nm}{ct}",
                                   tag="sq")
                    # q: scale=16 folds the Dh^-0.5: (16 x)^2 = 256 x^2
                    nc.scalar.activation(sqt[:], dst[:], AF.Square,
                                         scale=16.0 if ti == 0 else 1.0)
                    sq_tiles[(nm, ct)] = sqt

            def l2_chain(ti, nm, dest, head):
                """Per-head l2norm: ones-matrix matmul broadcasts the
                per-token sum of squares (+eps via a K=1 matmul) to all
                partitions; rsq = sqrt(1/ss) applied per token block."""
                er = epsrow[:, 0:TOKB] if ti == 0 else epsrow[:, TOKB:2 * TOKB]
                while ((nm, head * 2) not in sq_tiles
                       or (nm, head * 2 + 1) not in sq_tiles):
                    yield  # feeder proj chains still emitting
                for nb in range(NB):
                    bc = psw.tile([128, TOKB], fp32,
                                  name=f"bc{nm}{head}{nb}", tag="w")
                    nc.tensor.matmul(bc[:], ones128[0:1, :], er,
                                     start=True, stop=False)
                    for cth in range(2):
                        nc.tensor.matmul(
                            bc[:], ones128,
                            sq_tiles[(nm, head * 2 + cth)][
                                :, nb * TOKB:(nb + 1) * TOKB],
                            start=False, stop=(cth == 1))
                    bcs = bcp.tile([128, TOKB], bf16,
                                   name=f"bcs{nm}{head}{nb}", tag="bcs")
                    nc.scalar.copy(bcs[:], bc[:])
                    bcf = bcp.tile([128, TOKB], fp32,
                                   name=f"bcf{nm}{head}{nb}", tag="bcf")
                    nc.vector.reciprocal(bcf[:], bcs[:])
                    bcb = bcp.tile([128, TOKB], bf16,
                                   name=f"bcb{nm}{head}{nb}", tag="bcb")
                    nc.scalar.activation(bcb[:], bcf[:], AF.Sqrt)
                    sl = slice(nb * TOKB, (nb + 1) * TOKB)
                    for cth in range(2):
                        ct = head * 2 + cth
                        nc.vector.tensor_mul(dest[ct][:, sl],
                                             dest[ct][:, sl], bcb[:])
                    yield

            # head-0 chains first so the recurrence can start early; the
            # merged driver below staggers entry so concurrent chains sit
            # at different pipeline stages.
            aq = []
            for hd in range(2):
                c0 = hd * 2
                for ti, (nm, dest) in enumerate(
                        (("q", qh), ("k", kh), ("v", vh))):
                    aq.append((proj_chain(ti, nm, dest, c0), hd))
                    aq.append((proj_chain(ti, nm, dest, c0 + 1), hd))
                aq.append((l2_chain(0, "q", qh, hd), hd))
                aq.append((l2_chain(1, "k", kh, hd), hd))

            # ====== phase B + C: delta-rule recurrence, interleaved =====
            s_ps, s_sb = [], []
            for head in range(2):
                s_ps.append(pss.tile([128, 512], fp32, name=f"sps{head}",
                                     tag="sps"))
                t_ = recp.tile([128, 512], bf16, name=f"ssb{head}", tag="ssb",
                               bufs=6)
                nc.vector.memset(t_[:], 0.0)
                s_sb.append(t_)
            oTch = {}
            s_done = [-1, -1]

            def chunk_head(ch, head):
                """Generator emitting one head's chunk ops; yields between
                pipeline stages so the two heads' streams interleave."""
                t0 = ch * C
                ct0 = head * 2
                QT = [qh[ct0][:, t0:t0 + C], qh[ct0 + 1][:, t0:t0 + C]]
                KTt = [kh[ct0][:, t0:t0 + C], kh[ct0 + 1][:, t0:t0 + C]]
                VT = [vh[ct0][:, t0:t0 + C], vh[ct0 + 1][:, t0:t0 + C]]

                # K, V in [C, Dh] layout via PE transpose (bf16 psum)
                ptkv = psw.tile([128, 512], bf16, name=f"ptkv{head}{ch}",
                                tag="w")
                for i in range(2):
                    nc.tensor.transpose(ptkv[:, i * 128:(i + 1) * 128],
                                        KTt[i], ident)
                    nc.tensor.transpose(
                        ptkv[:, 256 + i * 128:256 + (i + 1) * 128],
                        VT[i], ident)
                kvcd = recp.tile([128, 512], bf16, name=f"kvcd{head}{ch}",
                                 tag="kvcd", bufs=4)
                if head == 0:
                    nc.scalar.copy(kvcd[:], ptkv[:])
                else:
                    nc.vector.tensor_copy(kvcd[:], ptkv[:])
                yield

                # KK^T / KQ^T share one psum bank; masked pieces in SBUF bf16
                pkx = psw.tile([128, 256], fp32, name=f"pkx{head}{ch}",
                               tag="w")
                pkk, pkq = pkx[:, 0:128], pkx[:, 128:256]
                for i in range(2):
                    nc.tensor.matmul(pkk, KTt[i], KTt[i], start=(i == 0),
                                     stop=(i == 1))
                for i in range(2):
                    nc.tensor.matmul(pkq, KTt[i], QT[i], start=(i == 0),
                                     stop=(i == 1))
                pkxS = recp.tile([128, 256], bf16, name=f"pkxS{head}{ch}",
                                 tag="pkxS", bufs=4)
                nc.scalar.copy(pkxS[:], pkx[:])
                pkkS, pkqS = pkxS[:, 0:128], pkxS[:, 128:256]
                yield

                Nl = recp.tile([128, 128], bf16, name=f"Nl{head}{ch}",
                               tag="Nl", bufs=4)
                Nu = recp.tile([128, 128], bf16, name=f"Nu{head}{ch}",
                               tag="Nu", bufs=4)
                R0 = recp.tile([128, 128], bf16, name=f"R0{head}{ch}",
                               tag="R0", bufs=4)
                Pat = recp.tile([128, 128], bf16, name=f"Pat{head}{ch}",
                                tag="Pat", bufs=4)
                nc.vector.tensor_mul(Nl[:], pkkS, m_bdl)
                nc.vector.tensor_mul(Nu[:], pkkS, m_bdu)
                nc.vector.tensor_sub(R0[:], ident, Nu[:])
                nc.vector.tensor_mul(Pat[:], pkqS, m_triuI)
                yield

                # R ~= (I+B)^{-1} = sum_{k<32} (-B)^k via Neumann doubling
                # (full 128x128 strict-triangular B; truncation err ~1e-5)
                pR = psw.tile([128, 128], fp32, name=f"pR{head}{ch}",
                              tag="w")
                nc.tensor.matmul(pR[:], ident, R0[:], start=True, stop=True)
                Rm, Pm, Qm = R0, Nl, Nu
                for lvl in range(4):
                    ppq = psw.tile([128, 256], fp32,
                                   name=f"ppq{head}{ch}{lvl}", tag="w")
                    nc.tensor.matmul(ppq[:, 0:128], Qm[:], Pm[:],
                                     start=True, stop=True)
                    PnQn = recp.tile([128, 256], bf16,
                                     name=f"PnQn{head}{ch}{lvl}", tag="PnQn",
                                     bufs=5)
                    Pn, Qn = PnQn[:, 0:128], PnQn[:, 128:256]
                    if lvl < 3:
                        nc.tensor.matmul(ppq[:, 128:256], Pm[:], Qm[:],
                                         start=True, stop=True)
                        if lvl % 2 == 0:
                            nc.vector.tensor_copy(PnQn[:], ppq[:])
                        else:
                            nc.scalar.copy(PnQn[:], ppq[:])
                    else:
                        nc.vector.tensor_copy(Pn, ppq[:, 0:128])
                        Qn = None
                    nc.tensor.matmul(pR[:], Pn, Rm[:], start=False,
                                     stop=True, skip_group_check=True)
                    Rn = recp.tile([128, 128], bf16,
                                   name=f"Rm{head}{ch}{lvl}", tag="Rm",
                                   bufs=5)
                    nc.scalar.copy(Rn[:], pR[:])
                    Rm, Pm, Qm = Rn, Pn, Qn
                    yield

                # gate: the S-dependent stage below must reference the S
                # tile produced by the previous chunk of this head
                while s_done[head] < ch - 1:
                    yield

                # RHS' = V - K S    (psum = K@S, then V - psum on DVE)
                pks = psw.tile([128, 256], fp32, name=f"pks{head}{ch}",
                               tag="w")
                for i in range(2):
                    nc.tensor.matmul(pks[:], KTt[i],
                                     s_sb[head][:, i * 256:(i + 1) * 256],
                                     start=(i == 0), stop=(i == 1))
                rhs_sb = recp.tile([128, 256], bf16, name=f"rhs{head}{ch}",
                                   tag="rhs", bufs=6)
                nc.vector.tensor_sub(rhs_sb[:], kvcd[:, 256:512], pks[:])
                yield

                # U' = R^T RHS'  (one matmul, no forward substitution)
                pu = psw.tile([128, 256], fp32, name=f"pu{head}{ch}",
                              tag="w")
                nc.tensor.matmul(pu[:], Rm[:], rhs_sb[:], start=True,
                                 stop=True)
                u_sb = recp.tile([128, 256], bf16, name=f"u{head}{ch}",
                                 tag="u", bufs=6)
                nc.vector.tensor_copy(u_sb[:], pu[:])
                yield

                # O = Q S + P^T U'
                po = psw.tile([128, 256], fp32, name=f"po{head}{ch}",
                              tag="w")
                for i in range(2):
                    nc.tensor.matmul(po[:], QT[i],
                                     s_sb[head][:, i * 256:(i + 1) * 256],
                                     start=(i == 0), stop=False)
                nc.tensor.matmul(po[:], Pat[:], u_sb[:], start=False,
                                 stop=True)

                # S += K^T U'   (accumulate in persistent psum); the last
                # chunk's updated S is never read again, so skip its update
                if ch < NCHUNK - 1:
                    for i in range(2):
                        nc.tensor.matmul(s_ps[head][:, i * 256:(i + 1) * 256],
                                         kvcd[:, i * 128:(i + 1) * 128],
                                         u_sb[:],
                                         start=(ch == 0 and i == 0), stop=True,
                                         skip_group_check=True)
                    s_nb = recp.tile([128, 512], bf16, name=f"ssb{head}{ch}",
                                     tag="ssb", bufs=6)
                    nc.scalar.copy(s_nb[:, 0:256], s_ps[head][:, 0:256])
                    nc.vector.tensor_copy(s_nb[:, 256:512],
                                          s_ps[head][:, 256:512])
                    s_sb[head] = s_nb
                s_done[head] = ch
                yield

                # RMSNorm rows of O, then transpose out
                osq = recp.tile([128, 256], bf16, name=f"osq{head}{ch}",
                                tag="osq", bufs=4)
                ossq = recp.tile([128, 1], fp32, name=f"ossq{head}{ch}",
                                 tag="ossq", bufs=4)
                nc.scalar.activation(osq[:], po[:], AF.Square,
                                     accum_out=ossq[:])
                orsq = recp.tile([128, 1], fp32, name=f"orsq{head}{ch}",
                                 tag="orsq", bufs=4)
                nc.scalar.activation(orsq[:], ossq[:], AF.Sqrt,
                                     bias=biases[:, 1:2], scale=1.0 / DH)
                nc.vector.reciprocal(orsq[:], orsq[:])
                onrm = recp.tile([128, 256], bf16, name=f"onrm{head}{ch}",
                                 tag="onrm", bufs=4)
                nc.scalar.activation(onrm[:], po[:], AF.Copy, scale=orsq[:])
                yield

                pto = psw.tile([128, 256], bf16, name=f"pto{head}{ch}",
                               tag="w")
                for i in range(2):
                    nc.tensor.transpose(pto[:, i * 128:(i + 1) * 128],
                                        onrm[:, i * 128:(i + 1) * 128],
                                        ident)
                oTt = otp.tile([128, 256], bf16, name=f"oT{head}{ch}",
                               tag="oT", bufs=6)
                nc.vector.tensor_copy(oTt[:], pto[:])
                oTch[(ch, head)] = oTt

            def out_proj(ch):
                """Output projection for one chunk (both heads)."""
                for half in range(2):
                    pf = psw.tile([128, 512], fp32, name=f"pf{ch}{half}",
                                  tag="w")
                    k = 0
                    for hd in range(2):
                        for i in range(2):
                            nc.tensor.matmul(
                                pf[:], oTch[(ch, hd)][:, i * 128:(i + 1) * 128],
                                wo_s[hd * 2 + i][:, half * 512:(half + 1) * 512],
                                start=(k == 0), stop=(k == 3))
                            k += 1
                    of = ofp.tile([128, 512], fp32, name=f"of{ch}{half}",
                                  tag="of")
                    if half == 0:
                        nc.scalar.copy(of[:], pf[:])
                    else:
                        nc.vector.tensor_copy(of[:], pf[:])
                    nc.sync.dma_start(
                        out_t[ch][:, half * 512:(half + 1) * 512], of[:])

            # merged driver: phase A chains (staggered entry) plus, per head,
            # up to 2 chunk-head generators (current + lookahead into the
            # next chunk, whose S-independent work fills the S-chain stalls).
            # A head's recurrence is admitted as soon as that head's
            # projection chains all finished, overlapping the other head's
            # phase A work.
            bqs = {h: [(chunk_head(c, h), c, h) for c in range(NCHUNK)]
                   for h in range(2)}
            a_left = {0: 8, 1: 8}
            active = []          # entries: (gen, kind, c, h)
            next_c = 0
            finished = set()
            rounds = 0
            while aq or bqs[0] or bqs[1] or active:
                a_act = sum(1 for e in active if e[1] == "a")
                if aq and (a_act < 2 or (rounds % 2 == 0 and a_act < 5)):
                    g, h = aq.pop(0)
                    active.append((g, "a", -1, h))
                for h in range(2):
                    b_act = sum(1 for e in active
                                if e[1] == "b" and e[3] == h)
                    if bqs[h] and b_act < 2 and a_left[h] == 0:
                        g, c, _ = bqs[h].pop(0)
                        active.append((g, "b", c, h))
                rounds += 1
                for ent in list(active):
                    g, kind, c, h = ent
                    try:
                        next(g)
                    except StopIteration:
                        active.remove(ent)
                        if kind == "a":
                            a_left[h] -= 1
                        else:
                            finished.add((c, h))
                while (not DEBUG_SKIP_WO and next_c < NCHUNK
                       and (next_c, 0) in finished
                       and (next_c, 1) in finished):
                    out_proj(next_c)
                    next_c += 1


LP_NP = np.float16  # host-side 16-bit dtype matching the device dtype


def _make_consts():
    ii = np.arange(128)
    ident = np.eye(128, dtype=np.float32)
    bdl = (ii[:, None] > ii[None, :]).astype(np.float32)
    bdu = (ii[:, None] < ii[None, :]).astype(np.float32)
    fneg = np.zeros((128, 128), np.float32)
    triuI = (ii[:, None] <= ii[None, :]).astype(np.float32)
    ones = np.ones((128, 128), np.float32)
    return np.concatenate([ident, bdl, bdu, fneg, triuI, ones],
                          axis=1).astype(LP_NP)


def _get_compiled():
    key = ("nc", SILU_NATIVE)
    if key not in _CACHE:
        _CACHE[key] = _build_bass()
    return _CACHE[key]


def kernel(hidden_states, Wq, Wk, Wv, conv_wq, conv_wk, conv_wv, onorm_w, Wo):
    from concourse.bass_utils import run_bass_kernel_spmd

    hidden_states = np.asarray(hidden_states, np.float32)
    Wq = np.asarray(Wq, np.float32)
    Wk = np.asarray(Wk, np.float32)
    Wv = np.asarray(Wv, np.float32)
    Wo = np.asarray(Wo, np.float32)
    conv_wq = np.asarray(conv_wq, np.float32)
    conv_wk = np.asarray(conv_wk, np.float32)
    conv_wv = np.asarray(conv_wv, np.float32)
    onorm_w = np.asarray(onorm_w, np.float32)

    bf = LP_NP
    consts = _make_consts()
    Wo_eff = (Wo * np.tile(onorm_w, H)[:, None]).astype(bf)  # fold RMS weight

    in_maps = []
    for core in range(NCORES):
        b, g = divmod(core, 2)
        cols = slice(CG * g, CG * (g + 1))
        in_maps.append({
            "xT": np.ascontiguousarray(hidden_states[b].T).astype(bf),
            "wq": np.ascontiguousarray(Wq[:, cols]).astype(bf),
            "wk": np.ascontiguousarray(Wk[:, cols]).astype(bf),
            "wv": np.ascontiguousarray(Wv[:, cols]).astype(bf),
            "wo": np.ascontiguousarray(Wo_eff[cols, :]),
            "cw": np.ascontiguousarray(np.concatenate(
                [conv_wq[cols], conv_wk[cols], conv_wv[cols]], axis=1)),
            "consts": consts,
        })

    nc = _get_compiled()
    res = run_bass_kernel_spmd(nc, in_maps, core_ids=list(range(NCORES)),
                               **_CACHE.get("run_kwargs", {}))
    _CACHE["last_results"] = res
    out = np.zeros((B, T, D), np.float32)
    for core in range(NCORES):
        out[core // 2] += res.results[core]["out"]
    return out
